# revision 19
# baseline (speedup 1.0000x reference)
"""Trainium2 Bass kernel for nn_Attn_VarLevel (sparse per-variable attention).

Math restructuring (exact, not approximate):
  reference:
    q  = queries @ Wq.T + bq                     [B,P,V,D]
    k  = keys @ Wkv.T + bkv                      [B,T,V,D]
    kc[b,p,v,n] = k[b, 32+p, c[b,v,n]]           (indices shared across p!)
    attn = softmax_n(q . kc / sqrt(D))
    out  = sum_n attn * kc
    y = concat(k[:, :32], out) @ Wout.T + bout

  kernel (zero-bias fast path; biases are zeros per the spec):
    * scores: G[v,u] = <q_v, k_u> = rawq_v . km_u with km = keys @ (Wq.T Wkv).T
      -- one key-side projection, no query projection at all.
    * duplicates in the index list are handled exactly by a multiplicity
      matrix mult[u,v] = #{n : c[v,n]==u}: softmax over n == masked softmax
      over u weighted by mult.
    * output projection folds into the keys (softmax weights sum to 1):
      kp = keys @ (Wkv.T Wout.T); y[t<32] = kp directly, y[t>=32] = attnw @ kp.
    * softmax denominator Z comes free as a 129th "ones" column of kp in the
      weighted-sum matmul; division is a per-partition scalar multiply.

Sharding: data-parallel over batch, 2 batches per core on 8 cores.
"""

import sys

sys.path.insert(0, "/opt/trn_rl_repo")

import numpy as np

import concourse.bass as bass
import concourse.bacc as bacc
import concourse.mybir as mybir
import concourse.tile as tile
from concourse.bass_utils import run_bass_kernel_spmd
from concourse.masks import make_identity

B, P, T, V, N, D = 16, 96, 128, 64, 16, 128
NCORES = 8
BPC = B // NCORES          # batches per core
QTOK = P * V               # 6144 query tokens per batch
KTOK = T * V               # 8192 key tokens per batch
KTILES = KTOK // 128       # 64
NCHUNK = 512               # matmul moving free dim
SCALE = float(D) ** -0.5

F32 = mybir.dt.float32

_cache = {}


def _build(reps=1):
    key = ("nc", reps)
    if key in _cache:
        return _cache[key]

    nc = bacc.Bacc(None, target_bir_lowering=False, debug=False)

    q_d = nc.declare_dram_parameter("queries", [BPC, QTOK, D], F32, isOutput=False)
    k_d = nc.declare_dram_parameter("keys", [BPC, KTOK, D], F32, isOutput=False)
    mb_d = nc.declare_dram_parameter("maskblk", [BPC, 128, 128], F32, isOutput=False)
    wqk_d = nc.declare_dram_parameter("wqk_t", [D, D], F32, isOutput=False)
    wfold_d = nc.declare_dram_parameter("wfold", [D, D], F32, isOutput=False)
    out_d = nc.declare_dram_parameter("out", [BPC, KTOK, D], F32, isOutput=True)

    with tile.TileContext(nc) as tc:
        with (
            tc.tile_pool(name="const", bufs=1) as constp,
            tc.tile_pool(name="raw", bufs=8) as rawp,
            tc.tile_pool(name="chunkT", bufs=3) as chunkp,
            tc.tile_pool(name="perm", bufs=2) as permp,
            tc.tile_pool(name="at", bufs=4) as atp,
            tc.tile_pool(name="y", bufs=6) as yp,
            tc.tile_pool(name="rz", bufs=6) as rzp,
            tc.tile_pool(name="ps_t", bufs=2, space=bass.MemorySpace.PSUM) as ps_t,
            tc.tile_pool(name="ps_p", bufs=2, space=bass.MemorySpace.PSUM) as ps_p,
            tc.tile_pool(name="ps_g", bufs=2, space=bass.MemorySpace.PSUM) as ps_g,
            tc.tile_pool(name="ps_ws", bufs=2, space=bass.MemorySpace.PSUM) as ps_ws,
        ):
            ident = constp.tile([128, 128], F32, tag="ident")
            make_identity(nc, ident[:])
            wqk_sb = constp.tile([D, D], F32, tag="wqk")
            wfold_sb = constp.tile([D, D], F32, tag="wfold")
            nc.sync.dma_start(wqk_sb[:], wqk_d[:])
            nc.sync.dma_start(wfold_sb[:], wfold_d[:])

            for bi in [b for _ in range(reps) for b in range(BPC)]:
                # persistent per-batch tensors
                rawqT = permp.tile([D, QTOK], F32, tag="rawqT")   # raw queries^T
                kmT = permp.tile([D, QTOK], F32, tag="kmT")       # km^T (scores)
                kp = permp.tile([128, KTILES, D + 1], F32, tag="kp")
                mblk = permp.tile([128, 128], F32, tag="mblk")
                nc.sync.dma_start(mblk[:], mb_d[bi])
                nc.vector.memset(kp[:, :, D : D + 1], 1.0)

                # ---- keys: transpose -> kp proj (+direct out t<32), km proj
                for c in range(KTOK // NCHUNK):          # 16 chunks of 512 tokens
                    ksT = chunkp.tile([128, NCHUNK], F32, tag="ksT")
                    pt = ps_t.tile([128, NCHUNK], F32, tag="pt")
                    # one 256KB strided DMA: [512 tok, 128] -> [128, 4, 128]
                    raw = rawp.tile([128, 4, 128], F32, tag="raw")
                    nc.sync.dma_start(
                        raw[:],
                        k_d[bi, c * NCHUNK : (c + 1) * NCHUNK, :].rearrange(
                            "(j p) d -> p j d", p=128
                        ),
                    )
                    for j in range(4):
                        nc.tensor.transpose(
                            pt[:, j * 128 : (j + 1) * 128], raw[:, j, :], ident[:]
                        )
                    nc.vector.tensor_copy(ksT[:], pt[:])
                    pp = ps_p.tile([128, NCHUNK], F32, tag="pp")
                    for j in range(4):
                        nc.tensor.matmul(
                            pp[:, j * 128 : (j + 1) * 128],
                            ksT[:, j * 128 : (j + 1) * 128],
                            wfold_sb[:],
                            start=True, stop=True,
                        )
                    if c < 4:
                        y4 = yp.tile([128, NCHUNK], F32, tag="y4")
                        nc.vector.tensor_copy(y4[:], pp[:])
                        nc.scalar.dma_start(
                            out_d[bi, c * NCHUNK : (c + 1) * NCHUNK, :].rearrange(
                                "(j p) d -> p j d", p=128
                            ),
                            y4[:].rearrange("p (j d) -> p j d", d=128),
                        )
                    else:
                        nc.scalar.copy(kp[:, c * 4 : c * 4 + 4, 0:D], pp[:])
                        pk = ps_p.tile([128, NCHUNK], F32, tag="pp")
                        nc.tensor.matmul(pk[:], wqk_sb[:], ksT[:], start=True, stop=True)
                        nc.scalar.copy(
                            kmT[:, (c - 4) * NCHUNK : (c - 3) * NCHUNK], pk[:]
                        )

                # ---- queries: transpose only (no projection needed)
                for c in range(QTOK // NCHUNK):          # 12 chunks
                    pt = ps_t.tile([128, NCHUNK], F32, tag="pt")
                    raw = rawp.tile([128, 4, 128], F32, tag="raw")
                    nc.sync.dma_start(
                        raw[:],
                        q_d[bi, c * NCHUNK : (c + 1) * NCHUNK, :].rearrange(
                            "(j p) d -> p j d", p=128
                        ),
                    )
                    for j in range(4):
                        nc.tensor.transpose(
                            pt[:, j * 128 : (j + 1) * 128], raw[:, j, :], ident[:]
                        )
                    nc.vector.tensor_copy(
                        rawqT[:, c * NCHUNK : (c + 1) * NCHUNK], pt[:]
                    )

                # ---- attention: per twin (2 positions share a 128-row tile)
                for tw in range(P // 2):
                    p0 = tw * 2
                    # full 128x128 gram for the twin; cross-pair blocks are
                    # zeroed by the block-diagonal mask below.
                    gps = ps_g.tile([128, 128], F32, tag="g")
                    nc.tensor.matmul(
                        gps[:],
                        kmT[:, p0 * 64 : (p0 + 2) * 64],
                        rawqT[:, p0 * 64 : (p0 + 2) * 64],
                        start=True, stop=True,
                    )
                    aT = atp.tile([128, 128], F32, tag="aT")
                    nc.scalar.activation(
                        aT[:], gps[:], mybir.ActivationFunctionType.Exp, scale=SCALE
                    )
                    nc.vector.tensor_mul(aT[:], aT[:], mblk[:])
                    # weighted sum (+ Z in col 128); masked zeros kill the
                    # cross-pair contributions exactly.
                    ti0 = (32 + p0) // 2
                    ws = ps_ws.tile([128, D + 1], F32, tag="ws")
                    nc.tensor.matmul(
                        ws[:], aT[:], kp[:, ti0, :], start=True, stop=True
                    )
                    rz = rzp.tile([128, 1], F32, tag="rz")
                    nc.vector.reciprocal(rz[:], ws[:, D : D + 1])
                    if tw % 2 == 0:
                        y2 = yp.tile([128, 2, 128], F32, tag="y")
                    nc.vector.tensor_scalar_mul(y2[:, tw % 2, :], ws[:, 0:D], rz[:])
                    if tw % 2 == 1:
                        tok0 = (32 + p0 - 2) * 64
                        nc.scalar.dma_start(
                            out_d[bi, tok0 : tok0 + 256, :].rearrange(
                                "(j p) d -> p j d", p=128
                            ),
                            y2[:],
                        )

    nc.finalize()
    _cache[key] = nc
    return nc


def prepare_in_maps(queries, keys, var_ccc, Wq, bq, Wkv, bkv, Wout, bout):
    queries = np.ascontiguousarray(np.asarray(queries, dtype=np.float32))
    keys = np.ascontiguousarray(np.asarray(keys, dtype=np.float32))
    var_ccc = np.asarray(var_ccc)
    Wq = np.asarray(Wq, dtype=np.float32)
    Wkv = np.asarray(Wkv, dtype=np.float32)
    Wout = np.asarray(Wout, dtype=np.float32)

    # multiplicity matrices: mult[b][u, v] = #{n : var_ccc[b,v,n] == u}
    mult = np.zeros((B, V, V), dtype=np.float32)
    vv = np.repeat(np.arange(V), N)
    for b in range(B):
        np.add.at(mult[b], (var_ccc[b].reshape(-1).astype(np.int64), vv), 1.0)
    # block-diagonal mask for a twin (2 positions) of gram blocks
    maskblk = np.zeros((B, 128, 128), dtype=np.float32)
    maskblk[:, 0:V, 0:V] = mult
    maskblk[:, V : 2 * V, V : 2 * V] = mult

    wqk_t = np.ascontiguousarray((Wq.T @ Wkv).T)         # lhsT for km proj
    wfold = np.ascontiguousarray(Wkv.T @ Wout.T)         # keys -> kp

    in_maps = []
    for c in range(NCORES):
        sl = slice(c * BPC, (c + 1) * BPC)
        in_maps.append(
            {
                "queries": queries[sl].reshape(BPC, QTOK, D),
                "keys": keys[sl].reshape(BPC, KTOK, D),
                "maskblk": maskblk[sl],
                "wqk_t": wqk_t,
                "wfold": wfold,
            }
        )
    return in_maps


def assemble_out(res):
    return np.concatenate(
        [res.results[c]["out"].reshape(BPC, T, V, D) for c in range(NCORES)], axis=0
    )


def _zero_bias(bq, bkv, bout):
    return (
        not np.any(np.asarray(bq)) and not np.any(np.asarray(bkv))
        and not np.any(np.asarray(bout))
    )


def _numpy_fallback(queries, keys, var_ccc, Wq, bq, Wkv, bkv, Wout, bout):
    # exact host fallback for the (spec-impossible) nonzero-bias case
    queries = np.asarray(queries, np.float64)
    keys = np.asarray(keys, np.float64)
    b, p, v, d = queries.shape
    q = queries @ Wq.T + bq
    k = keys @ Wkv.T + bkv
    k_last = k[:, -p:]
    idx = np.asarray(var_ccc).reshape(b, -1)
    kc = np.stack([k_last[i][:, idx[i]] for i in range(b)]).reshape(b, p, v, -1, d)
    s = np.einsum("bpvd,bpvnd->bpvn", q, kc) * (d ** -0.5)
    e = np.exp(s - s.max(-1, keepdims=True))
    attn = e / e.sum(-1, keepdims=True)
    out = np.einsum("bpvn,bpvnd->bpvd", attn, kc)
    res = np.concatenate([k[:, :-p], out], axis=1)
    return (res @ Wout.T + bout).astype(np.float32)


def kernel(**inputs):
    if not _zero_bias(inputs["bq"], inputs["bkv"], inputs["bout"]):
        return _numpy_fallback(**inputs)
    nc = _build()
    in_maps = prepare_in_maps(**inputs)
    res = run_bass_kernel_spmd(nc, in_maps, list(range(NCORES)))
    return assemble_out(res)


# revision 20
# speedup vs baseline: 1.0597x; 1.0597x over previous
"""Trainium2 Bass kernel for nn_Attn_VarLevel (sparse per-variable attention).

Math restructuring (exact, not approximate):
  reference:
    q  = queries @ Wq.T + bq                     [B,P,V,D]
    k  = keys @ Wkv.T + bkv                      [B,T,V,D]
    kc[b,p,v,n] = k[b, 32+p, c[b,v,n]]           (indices shared across p!)
    attn = softmax_n(q . kc / sqrt(D))
    out  = sum_n attn * kc
    y = concat(k[:, :32], out) @ Wout.T + bout

  kernel (zero-bias fast path; biases are zeros per the spec):
    * scores: G[v,u] = <q_v, k_u> = rawq_v . km_u with km = keys @ (Wq.T Wkv).T
      -- one key-side projection, no query projection at all.
    * duplicates in the index list are handled exactly by a multiplicity
      matrix mult[u,v] = #{n : c[v,n]==u}: softmax over n == masked softmax
      over u weighted by mult.  Two positions p share one 128x128 gram
      matmul; the block-diagonal mask zeroes the cross-position blocks, so
      one weighted-sum matmul per twin is exact.
    * output projection folds into the keys (softmax weights sum to 1):
      kp = keys @ (Wkv.T Wout.T); y[t<32] = kp directly, y[t>=32] = attnw @ kp.
    * softmax denominator Z comes free as a 129th "ones" column of kp in the
      weighted-sum matmul; division is a per-partition scalar multiply.
    * queries/keys are transposed to [D, token] on the host so the kernel
      DMAs directly into the layout the tensor engine needs (no on-chip
      transposes at all).

Sharding: data-parallel over batch, 2 batches per core on 8 cores.
"""

import sys

sys.path.insert(0, "/opt/trn_rl_repo")

import numpy as np

import concourse.bass as bass
import concourse.bacc as bacc
import concourse.mybir as mybir
import concourse.tile as tile
from concourse.bass_utils import run_bass_kernel_spmd

B, P, T, V, N, D = 16, 96, 128, 64, 16, 128
NCORES = 8
BPC = B // NCORES          # batches per core
QTOK = P * V               # 6144 query tokens per batch
KTOK = T * V               # 8192 key tokens per batch
KTILES = KTOK // 128       # 64
NCHUNK = 512               # matmul moving free dim
SCALE = float(D) ** -0.5

F32 = mybir.dt.float32

_cache = {}


def _build(reps=1):
    key = ("nc", reps)
    if key in _cache:
        return _cache[key]

    nc = bacc.Bacc(None, target_bir_lowering=False, debug=False)

    qt_d = nc.declare_dram_parameter("queriesT", [BPC, D, QTOK], F32, isOutput=False)
    kt_d = nc.declare_dram_parameter("keysT", [BPC, D, KTOK], F32, isOutput=False)
    mb_d = nc.declare_dram_parameter("maskblk", [BPC, 128, 128], F32, isOutput=False)
    wqk_d = nc.declare_dram_parameter("wqk_t", [D, D], F32, isOutput=False)
    wfold_d = nc.declare_dram_parameter("wfold", [D, D], F32, isOutput=False)
    out_d = nc.declare_dram_parameter("out", [BPC, KTOK, D], F32, isOutput=True)

    with tile.TileContext(nc) as tc:
        with (
            tc.tile_pool(name="const", bufs=1) as constp,
            tc.tile_pool(name="chunkT", bufs=4) as chunkp,
            tc.tile_pool(name="perm", bufs=2) as permp,
            tc.tile_pool(name="at", bufs=6) as atp,
            tc.tile_pool(name="y", bufs=6) as yp,
            tc.tile_pool(name="rz", bufs=8) as rzp,
            tc.tile_pool(name="ps_p", bufs=2, space=bass.MemorySpace.PSUM) as ps_p,
            tc.tile_pool(name="ps_g", bufs=3, space=bass.MemorySpace.PSUM) as ps_g,
            tc.tile_pool(name="ps_ws", bufs=3, space=bass.MemorySpace.PSUM) as ps_ws,
        ):
            wqk_sb = constp.tile([D, D], F32, tag="wqk")
            wfold_sb = constp.tile([D, D], F32, tag="wfold")
            nc.sync.dma_start(wqk_sb[:], wqk_d[:])
            nc.sync.dma_start(wfold_sb[:], wfold_d[:])

            for bi in [b for _ in range(reps) for b in range(BPC)]:
                # persistent per-batch tensors
                rawqT = permp.tile([D, QTOK], F32, tag="rawqT")   # raw queries^T
                kmT = permp.tile([D, QTOK], F32, tag="kmT")       # km^T (scores)
                kp = permp.tile([128, KTILES, D + 1], F32, tag="kp")
                mblk = permp.tile([128, 128], F32, tag="mblk")
                nc.sync.dma_start(mblk[:], mb_d[bi])
                nc.sync.dma_start(rawqT[:], qt_d[bi])
                nc.vector.memset(kp[:, :, D : D + 1], 1.0)

                # ---- keys: kp proj (+direct out t<32), km proj
                for c in range(KTOK // NCHUNK):          # 16 chunks of 512 tokens
                    ksT = chunkp.tile([128, NCHUNK], F32, tag="ksT")
                    nc.sync.dma_start(
                        ksT[:], kt_d[bi, :, c * NCHUNK : (c + 1) * NCHUNK]
                    )
                    pp = ps_p.tile([128, NCHUNK], F32, tag="pp")
                    for j in range(4):
                        nc.tensor.matmul(
                            pp[:, j * 128 : (j + 1) * 128],
                            ksT[:, j * 128 : (j + 1) * 128],
                            wfold_sb[:],
                            start=True, stop=True,
                        )
                    if c < 4:
                        y4 = yp.tile([128, NCHUNK], F32, tag="y4")
                        nc.vector.tensor_copy(y4[:], pp[:])
                        nc.scalar.dma_start(
                            out_d[bi, c * NCHUNK : (c + 1) * NCHUNK, :].rearrange(
                                "(j p) d -> p j d", p=128
                            ),
                            y4[:].rearrange("p (j d) -> p j d", d=128),
                        )
                    else:
                        nc.scalar.copy(kp[:, c * 4 : c * 4 + 4, 0:D], pp[:])
                        pk = ps_p.tile([128, NCHUNK], F32, tag="pp")
                        nc.tensor.matmul(pk[:], wqk_sb[:], ksT[:], start=True, stop=True)
                        nc.scalar.copy(
                            kmT[:, (c - 4) * NCHUNK : (c - 3) * NCHUNK], pk[:]
                        )

                # ---- attention: per twin (2 positions share a 128-row tile)
                for tw in range(P // 2):
                    p0 = tw * 2
                    gps = ps_g.tile([128, 128], F32, tag="g")
                    nc.tensor.matmul(
                        gps[:],
                        kmT[:, p0 * 64 : (p0 + 2) * 64],
                        rawqT[:, p0 * 64 : (p0 + 2) * 64],
                        start=True, stop=True,
                    )
                    aT = atp.tile([128, 128], F32, tag="aT")
                    nc.scalar.activation(
                        aT[:], gps[:], mybir.ActivationFunctionType.Exp, scale=SCALE
                    )
                    nc.vector.tensor_mul(aT[:], aT[:], mblk[:])
                    ti0 = (32 + p0) // 2
                    ws = ps_ws.tile([128, D + 1], F32, tag="ws")
                    nc.tensor.matmul(
                        ws[:], aT[:], kp[:, ti0, :], start=True, stop=True
                    )
                    rz = rzp.tile([128, 1], F32, tag="rz")
                    nc.vector.reciprocal(rz[:], ws[:, D : D + 1])
                    if tw % 2 == 0:
                        y2 = yp.tile([128, 2, 128], F32, tag="y")
                    nc.vector.tensor_scalar_mul(y2[:, tw % 2, :], ws[:, 0:D], rz[:])
                    if tw % 2 == 1:
                        tok0 = (32 + p0 - 2) * 64
                        nc.scalar.dma_start(
                            out_d[bi, tok0 : tok0 + 256, :].rearrange(
                                "(j p) d -> p j d", p=128
                            ),
                            y2[:],
                        )

    nc.finalize()
    _cache[key] = nc
    return nc


def prepare_in_maps(queries, keys, var_ccc, Wq, bq, Wkv, bkv, Wout, bout):
    queries = np.asarray(queries, dtype=np.float32)
    keys = np.asarray(keys, dtype=np.float32)
    var_ccc = np.asarray(var_ccc)
    Wq = np.asarray(Wq, dtype=np.float32)
    Wkv = np.asarray(Wkv, dtype=np.float32)
    Wout = np.asarray(Wout, dtype=np.float32)

    # host-side transpose to the [D, token] layout the tensor engine wants
    queriesT = np.ascontiguousarray(queries.reshape(B, QTOK, D).transpose(0, 2, 1))
    keysT = np.ascontiguousarray(keys.reshape(B, KTOK, D).transpose(0, 2, 1))

    # multiplicity matrices: mult[b][u, v] = #{n : var_ccc[b,v,n] == u}
    mult = np.zeros((B, V, V), dtype=np.float32)
    vv = np.repeat(np.arange(V), N)
    for b in range(B):
        np.add.at(mult[b], (var_ccc[b].reshape(-1).astype(np.int64), vv), 1.0)
    # block-diagonal mask for a twin (2 positions) of gram blocks
    maskblk = np.zeros((B, 128, 128), dtype=np.float32)
    maskblk[:, 0:V, 0:V] = mult
    maskblk[:, V : 2 * V, V : 2 * V] = mult

    wqk_t = np.ascontiguousarray((Wq.T @ Wkv).T)         # lhsT for km proj
    wfold = np.ascontiguousarray(Wkv.T @ Wout.T)         # keys -> kp

    in_maps = []
    for c in range(NCORES):
        sl = slice(c * BPC, (c + 1) * BPC)
        in_maps.append(
            {
                "queriesT": queriesT[sl],
                "keysT": keysT[sl],
                "maskblk": maskblk[sl],
                "wqk_t": wqk_t,
                "wfold": wfold,
            }
        )
    return in_maps


def assemble_out(res):
    return np.concatenate(
        [res.results[c]["out"].reshape(BPC, T, V, D) for c in range(NCORES)], axis=0
    )


def _zero_bias(bq, bkv, bout):
    return (
        not np.any(np.asarray(bq)) and not np.any(np.asarray(bkv))
        and not np.any(np.asarray(bout))
    )


def _numpy_fallback(queries, keys, var_ccc, Wq, bq, Wkv, bkv, Wout, bout):
    # exact host fallback for the (spec-impossible) nonzero-bias case
    queries = np.asarray(queries, np.float64)
    keys = np.asarray(keys, np.float64)
    b, p, v, d = queries.shape
    q = queries @ Wq.T + bq
    k = keys @ Wkv.T + bkv
    k_last = k[:, -p:]
    idx = np.asarray(var_ccc).reshape(b, -1)
    kc = np.stack([k_last[i][:, idx[i]] for i in range(b)]).reshape(b, p, v, -1, d)
    s = np.einsum("bpvd,bpvnd->bpvn", q, kc) * (d ** -0.5)
    e = np.exp(s - s.max(-1, keepdims=True))
    attn = e / e.sum(-1, keepdims=True)
    out = np.einsum("bpvn,bpvnd->bpvd", attn, kc)
    res = np.concatenate([k[:, :-p], out], axis=1)
    return (res @ Wout.T + bout).astype(np.float32)


def kernel(**inputs):
    if not _zero_bias(inputs["bq"], inputs["bkv"], inputs["bout"]):
        return _numpy_fallback(**inputs)
    nc = _build()
    in_maps = prepare_in_maps(**inputs)
    res = run_bass_kernel_spmd(nc, in_maps, list(range(NCORES)))
    return assemble_out(res)


# revision 25
# speedup vs baseline: 1.0607x; 1.0010x over previous
"""Trainium2 Bass kernel for nn_Attn_VarLevel (sparse per-variable attention).

Math restructuring (exact, not approximate):
  reference:
    q  = queries @ Wq.T + bq                     [B,P,V,D]
    k  = keys @ Wkv.T + bkv                      [B,T,V,D]
    kc[b,p,v,n] = k[b, 32+p, c[b,v,n]]           (indices shared across p!)
    attn = softmax_n(q . kc / sqrt(D))
    out  = sum_n attn * kc
    y = concat(k[:, :32], out) @ Wout.T + bout

  kernel (zero-bias fast path; biases are zeros per the spec):
    * scores: G[v,u] = <q_v, k_u> = rawq_v . km_u with km = keys @ (Wq.T Wkv).T
      -- one key-side projection, no query projection at all.
    * duplicates in the index list are handled exactly by a multiplicity
      matrix mult[u,v] = #{n : c[v,n]==u}: softmax over n == masked softmax
      over u weighted by mult.  Two positions p share one 128x128 gram
      matmul; the block-diagonal mask zeroes the cross-position blocks, so
      one weighted-sum matmul per twin is exact.
    * output projection folds into the keys (softmax weights sum to 1):
      kp = keys @ (Wkv.T Wout.T); y[t<32] = kp directly, y[t>=32] = attnw @ kp.
    * softmax denominator Z comes free as a 129th "ones" column of kp in the
      weighted-sum matmul; division is a per-partition scalar multiply.
    * queries/keys are transposed to [D, token] on the host so the kernel
      DMAs directly into the layout the tensor engine needs (no on-chip
      transposes at all).

Sharding: data-parallel over batch, 2 batches per core on 8 cores.
"""

import sys

sys.path.insert(0, "/opt/trn_rl_repo")

import numpy as np

import concourse.bass as bass
import concourse.bacc as bacc
import concourse.mybir as mybir
import concourse.tile as tile
from concourse.bass_utils import run_bass_kernel_spmd

B, P, T, V, N, D = 16, 96, 128, 64, 16, 128
NCORES = 8
BPC = B // NCORES          # batches per core
QTOK = P * V               # 6144 query tokens per batch
KTOK = T * V               # 8192 key tokens per batch
KTILES = KTOK // 128       # 64
NCHUNK = 512               # matmul moving free dim
SCALE = float(D) ** -0.5

F32 = mybir.dt.float32

_cache = {}


def _build(reps=1):
    key = ("nc", reps)
    if key in _cache:
        return _cache[key]

    nc = bacc.Bacc(None, target_bir_lowering=False, debug=False)

    qt_d = nc.declare_dram_parameter("queriesT", [BPC, D, QTOK], F32, isOutput=False)
    kt_d = nc.declare_dram_parameter("keysT", [BPC, D, KTOK], F32, isOutput=False)
    mb_d = nc.declare_dram_parameter("maskblk", [BPC, 128, 128], F32, isOutput=False)
    wqk_d = nc.declare_dram_parameter("wqk_t", [D, D], F32, isOutput=False)
    wfold_d = nc.declare_dram_parameter("wfold", [D, D], F32, isOutput=False)
    out_d = nc.declare_dram_parameter("out", [BPC, KTOK, D], F32, isOutput=True)

    with tile.TileContext(nc) as tc:
        with (
            tc.tile_pool(name="const", bufs=1) as constp,
            tc.tile_pool(name="chunkT", bufs=4) as chunkp,
            tc.tile_pool(name="perm", bufs=2) as permp,
            tc.tile_pool(name="at", bufs=6) as atp,
            tc.tile_pool(name="y", bufs=6) as yp,
            tc.tile_pool(name="rz", bufs=8) as rzp,
            tc.tile_pool(name="ps_p", bufs=2, space=bass.MemorySpace.PSUM) as ps_p,
            tc.tile_pool(name="ps_g", bufs=3, space=bass.MemorySpace.PSUM) as ps_g,
            tc.tile_pool(name="ps_ws", bufs=3, space=bass.MemorySpace.PSUM) as ps_ws,
        ):
            wqk_sb = constp.tile([D, D], F32, tag="wqk")
            wfold_sb = constp.tile([D, D], F32, tag="wfold")
            nc.sync.dma_start(wqk_sb[:], wqk_d[:])
            nc.sync.dma_start(wfold_sb[:], wfold_d[:])

            for bi in [b for _ in range(reps) for b in range(BPC)]:
                # persistent per-batch tensors
                rawqT = permp.tile([D, QTOK], F32, tag="rawqT")   # raw queries^T
                kmT = permp.tile([D, QTOK], F32, tag="kmT")       # km^T (scores)
                kp = permp.tile([128, KTILES, D + 1], F32, tag="kp")
                mblk = permp.tile([128, 128], F32, tag="mblk")
                nc.sync.dma_start(mblk[:], mb_d[bi])
                nc.sync.dma_start(rawqT[:], qt_d[bi])
                nc.vector.memset(kp[:, :, D : D + 1], 1.0)

                # ---- keys: kp proj (+direct out t<32), km proj; attention
                # twins are emitted as soon as their kp/km chunks are ready
                # so the scheduler pipelines the two phases.
                def key_chunk(c):
                    ksT = chunkp.tile([128, NCHUNK], F32, tag="ksT")
                    nc.sync.dma_start(
                        ksT[:], kt_d[bi, :, c * NCHUNK : (c + 1) * NCHUNK]
                    )
                    pp = ps_p.tile([128, NCHUNK], F32, tag="pp")
                    for j in range(4):
                        nc.tensor.matmul(
                            pp[:, j * 128 : (j + 1) * 128],
                            ksT[:, j * 128 : (j + 1) * 128],
                            wfold_sb[:],
                            start=True, stop=True,
                        )
                    if c < 4:
                        y4 = yp.tile([128, NCHUNK], F32, tag="y4")
                        nc.vector.tensor_copy(y4[:], pp[:])
                        nc.scalar.dma_start(
                            out_d[bi, c * NCHUNK : (c + 1) * NCHUNK, :].rearrange(
                                "(j p) d -> p j d", p=128
                            ),
                            y4[:].rearrange("p (j d) -> p j d", d=128),
                        )
                    else:
                        nc.scalar.copy(kp[:, c * 4 : c * 4 + 4, 0:D], pp[:])
                        pk = ps_p.tile([128, NCHUNK], F32, tag="pp")
                        nc.tensor.matmul(pk[:], wqk_sb[:], ksT[:], start=True, stop=True)
                        nc.scalar.copy(
                            kmT[:, (c - 4) * NCHUNK : (c - 3) * NCHUNK], pk[:]
                        )

                _state = {}

                def twin(tw):
                    p0 = tw * 2
                    gps = ps_g.tile([128, 128], F32, tag="g")
                    nc.tensor.matmul(
                        gps[:],
                        kmT[:, p0 * 64 : (p0 + 2) * 64],
                        rawqT[:, p0 * 64 : (p0 + 2) * 64],
                        start=True, stop=True,
                    )
                    aT = atp.tile([128, 128], F32, tag="aT")
                    nc.scalar.activation(
                        aT[:], gps[:], mybir.ActivationFunctionType.Exp, scale=SCALE
                    )
                    nc.vector.tensor_mul(aT[:], aT[:], mblk[:])
                    ti0 = (32 + p0) // 2
                    ws = ps_ws.tile([128, D + 1], F32, tag="ws")
                    nc.tensor.matmul(
                        ws[:], aT[:], kp[:, ti0, :], start=True, stop=True
                    )
                    rz = rzp.tile([128, 1], F32, tag="rz")
                    nc.vector.reciprocal(rz[:], ws[:, D : D + 1])
                    if tw % 2 == 0:
                        y2 = yp.tile([128, 2, 128], F32, tag="y")
                        _state["y2"] = y2
                    y2 = _state["y2"]
                    nc.vector.tensor_scalar_mul(y2[:, tw % 2, :], ws[:, 0:D], rz[:])
                    if tw % 2 == 1:
                        tok0 = (32 + p0 - 2) * 64
                        nc.scalar.dma_start(
                            out_d[bi, tok0 : tok0 + 256, :].rearrange(
                                "(j p) d -> p j d", p=128
                            ),
                            y2[:],
                        )

                emitted = 0
                for c in range(KTOK // NCHUNK):          # 16 chunks of 512 tokens
                    key_chunk(c)
                    ready = min(max(4 * c - 12, 0), P // 2)
                    # keep twin pairs together so y2 stores stay merged
                    ready -= ready % 2
                    while emitted < ready:
                        twin(emitted)
                        emitted += 1
                while emitted < P // 2:
                    twin(emitted)
                    emitted += 1

    nc.finalize()
    _cache[key] = nc
    return nc


def prepare_in_maps(queries, keys, var_ccc, Wq, bq, Wkv, bkv, Wout, bout):
    queries = np.asarray(queries, dtype=np.float32)
    keys = np.asarray(keys, dtype=np.float32)
    var_ccc = np.asarray(var_ccc)
    Wq = np.asarray(Wq, dtype=np.float32)
    Wkv = np.asarray(Wkv, dtype=np.float32)
    Wout = np.asarray(Wout, dtype=np.float32)

    # host-side transpose to the [D, token] layout the tensor engine wants
    queriesT = np.ascontiguousarray(queries.reshape(B, QTOK, D).transpose(0, 2, 1))
    keysT = np.ascontiguousarray(keys.reshape(B, KTOK, D).transpose(0, 2, 1))

    # multiplicity matrices: mult[b][u, v] = #{n : var_ccc[b,v,n] == u}
    mult = np.zeros((B, V, V), dtype=np.float32)
    vv = np.repeat(np.arange(V), N)
    for b in range(B):
        np.add.at(mult[b], (var_ccc[b].reshape(-1).astype(np.int64), vv), 1.0)
    # block-diagonal mask for a twin (2 positions) of gram blocks
    maskblk = np.zeros((B, 128, 128), dtype=np.float32)
    maskblk[:, 0:V, 0:V] = mult
    maskblk[:, V : 2 * V, V : 2 * V] = mult

    wqk_t = np.ascontiguousarray((Wq.T @ Wkv).T)         # lhsT for km proj
    wfold = np.ascontiguousarray(Wkv.T @ Wout.T)         # keys -> kp

    in_maps = []
    for c in range(NCORES):
        sl = slice(c * BPC, (c + 1) * BPC)
        in_maps.append(
            {
                "queriesT": queriesT[sl],
                "keysT": keysT[sl],
                "maskblk": maskblk[sl],
                "wqk_t": wqk_t,
                "wfold": wfold,
            }
        )
    return in_maps


def assemble_out(res):
    return np.concatenate(
        [res.results[c]["out"].reshape(BPC, T, V, D) for c in range(NCORES)], axis=0
    )


def _zero_bias(bq, bkv, bout):
    return (
        not np.any(np.asarray(bq)) and not np.any(np.asarray(bkv))
        and not np.any(np.asarray(bout))
    )


def _numpy_fallback(queries, keys, var_ccc, Wq, bq, Wkv, bkv, Wout, bout):
    # exact host fallback for the (spec-impossible) nonzero-bias case
    queries = np.asarray(queries, np.float64)
    keys = np.asarray(keys, np.float64)
    b, p, v, d = queries.shape
    q = queries @ Wq.T + bq
    k = keys @ Wkv.T + bkv
    k_last = k[:, -p:]
    idx = np.asarray(var_ccc).reshape(b, -1)
    kc = np.stack([k_last[i][:, idx[i]] for i in range(b)]).reshape(b, p, v, -1, d)
    s = np.einsum("bpvd,bpvnd->bpvn", q, kc) * (d ** -0.5)
    e = np.exp(s - s.max(-1, keepdims=True))
    attn = e / e.sum(-1, keepdims=True)
    out = np.einsum("bpvn,bpvnd->bpvd", attn, kc)
    res = np.concatenate([k[:, :-p], out], axis=1)
    return (res @ Wout.T + bout).astype(np.float32)


def kernel(**inputs):
    if not _zero_bias(inputs["bq"], inputs["bkv"], inputs["bout"]):
        return _numpy_fallback(**inputs)
    nc = _build()
    in_maps = prepare_in_maps(**inputs)
    res = run_bass_kernel_spmd(nc, in_maps, list(range(NCORES)))
    return assemble_out(res)
